# revision 14
# baseline (speedup 1.0000x reference)
"""Trainium2 Bass kernel for the dynamic-kernel ECA module.

Computation per sample:
  gap  = mean(x, axis=l)                       (c,)
  h    = gelu(gap @ w1.T + b1)                 (hidden,)
  th   = tanh(h @ w2.T + b2); delta = 2*th     scalar
  k    = (5 + clip(round(delta), -3, 3)) | 1   in {3,5,7} (delta in (-2,2))
  w    = box filter of width k in 9-tap window, 1/k weights
  y    = conv1d(gap, w) along c (zero pad 4)   (c,)
  s    = sigmoid(y)
  out  = x * s[:, None]

Sharding: pure data parallel, batch 16 -> 8 cores x 2 samples.

Memory strategy (per core): x moves through HBM in bf16 (the 2e-2
rel-err budget gives ~10x margin over bf16's ~2e-3 quantization; the
kernel is purely HBM-bandwidth-bound at ~420 GB/s/core).  The 16 MiB
bf16 shard fits in SBUF entirely, so x is read exactly once and out
written exactly once (32 MiB/core of traffic ~= 80 us of DMA).

Queue discipline: the SP HWDGE queue carries ONLY the 16 x-tile loads
(an in-order sequencer; any gate-dependent wait interleaved there
head-of-line-blocks the remaining loads).  All constants ship as a
single packed transfer on the ScalarE HWDGE queue, which also carries
the 8 stores.  The first store is gated on the third-to-last load
(completion receipts lag data by a few us under full DMA load), giving
a pure read phase then a pure write phase.

The gate is built to minimize serial cross-engine hops, which at ~1 us
each (sem latency + engine-stream contention) dominate its latency:
PE runs the MLP matmuls AND all nine candidate band-conv matmuls
(k in {3,5,7} x in-chunk/hi-wrap/lo-wrap, host-precomputed constant
lhsT) back to back; ScalarE turns each candidate PSUM into
tanh(y_k/2); the scalar th is broadcast across partitions by a
ones-row matmul; VectorE then computes the three k-selection flags and
blends the three candidate results (sigmoid(y) = 0.5 + 0.5*tanh(y/2),
and sum(flags) = 1, so the affine folds into one op).  Nothing
data-dependent ever touches the matmul weights.

Load reductions (l-sums) are split per 1 MiB half-load and spread over
VectorE/ScalarE by explicit deadline order: sample-1's tail reductions
gate gate1 -> s1 scales -> s1 stores, so they get dedicated slots;
VectorE-assigned ones use a bf16 2x-mode fold-add before a half-size
1x reduce.  1/L is folded into w1 and the band weights on the host.
"""

import os
from contextlib import ExitStack

import numpy as np
import ml_dtypes

import concourse.bacc as bacc
import concourse.mybir as mybir
import concourse.tile as tile
from concourse.tile_rust import add_dep_helper
from concourse.bass_utils import run_bass_kernel_spmd

F32 = mybir.dt.float32
BF16 = mybir.dt.bfloat16
ALU = mybir.AluOpType
ACTF = mybir.ActivationFunctionType
AX_X = mybir.AxisListType.X

B, C, L = 16, 512, 8192
HID = 64
N_CORES = 8
BS = B // N_CORES            # samples per core = 2
CP = C // 128                # channel chunks = 4
NH = 2                       # load halves per tile
HL = L // NH                 # 4096 elements = 1 MiB bf16 per half-load

# packed const blob layout (f32 columns)
W1T_OFF = 0                  # [128, CP*HID] = 256 cols
WB_OFF = 256                 # [128, 3*3*128] = 1152 cols (m-major, k-minor)
B1_OFF = 1408                # [64, 1]
W2T_OFF = 1409               # [64, 1]
CST_COLS = 1410

S0_RED = ['v', 'a', 'v', 'a', 'v', 'a', 'v', 'a']


def _inst(x):
    return getattr(x, "ins", x)


def _build(b2_val):
    nc = bacc.Bacc("TRN2", target_bir_lowering=False, debug=False,
                   num_devices=N_CORES)

    x_d = nc.dram_tensor("x", [BS, C, L], BF16, kind="ExternalInput").ap()
    cst_d = nc.dram_tensor("cst", [128, CST_COLS], F32,
                           kind="ExternalInput").ap()
    o_d = nc.dram_tensor("out", [BS, C, L], BF16, kind="ExternalOutput").ap()

    with ExitStack() as ctx:
        tc = ctx.enter_context(tile.TileContext(nc))
        cache = ctx.enter_context(tc.tile_pool(name="cache", bufs=1))
        small = ctx.enter_context(tc.tile_pool(name="small", bufs=1))
        psum = ctx.enter_context(tc.tile_pool(name="psum", bufs=1, space="PSUM"))

        xt = {}
        partials = {}
        ge = {}
        loads = []

        def wb(m, kidx):
            o = WB_OFF + (m * 3 + kidx) * 128
            return cst[:, o:o + 128]

        def reduce_half(s, ci, h, eng):
            t = xt[(s, ci)][:, h * HL:(h + 1) * HL]
            dst = partials[s][:, ci, h:h + 1]
            if eng == 'v':
                nc.vector.reduce_sum(out=dst, in_=t, axis=AX_X)
            elif eng == 'vf':
                # bf16 fold-add at 2x mode, then a half-size 1x reduce
                f = small.tile([128, HL // 2], BF16, tag="fold")
                nc.vector.tensor_add(f[:], t[:, 0:HL // 2], t[:, HL // 2:HL])
                nc.vector.reduce_sum(out=dst, in_=f[:], axis=AX_X)
            else:
                nc.scalar.activation(t, t, ACTF.Copy, accum_out=dst)

        # ---- init + packed consts (single ScalarE-queue transfer) --------
        ge[0] = small.tile([128, CP + 2], F32, tag="ge0", name="ge0")
        ge[1] = small.tile([128, CP + 2], F32, tag="ge1", name="ge1")
        ones = small.tile([1, 128], F32, tag="ones")
        b2t = small.tile([1, 1], F32, tag="b2t")
        nc.vector.memset(ge[0][:], 0.0)
        nc.vector.memset(ge[1][:], 0.0)
        nc.vector.memset(ones[:], 1.0)
        nc.vector.memset(b2t[:], float(b2_val))
        cst = small.tile([128, CST_COLS], F32, tag="cst")
        nc.scalar.dma_start(out=cst[:], in_=cst_d[:])
        b1 = cst[0:HID, B1_OFF:B1_OFF + 1]
        w2t = cst[0:HID, W2T_OFF:W2T_OFF + 1]

        # s1's tail tiles (c3 both halves, c2 second half) load EARLY on
        # the ScalarE queue: completion receipts lag data by >10 us once
        # the store burst saturates DMA, so anything feeding gmean1 that
        # completes late delays gate1 -> s1 scales -> s1 stores.  Loading
        # them under light traffic gets their sems (and reductions) done
        # tens of us before they're needed.
        for ci in (2, 3):
            xt[(1, ci)] = cache.tile([128, L], BF16, tag=f"x1{ci}",
                                     name=f"x1{ci}")
        for ci, h in ((3, 0), (3, 1), (2, 1)):
            nc.scalar.dma_start(
                out=xt[(1, ci)][:, h * HL:(h + 1) * HL],
                in_=x_d[1, ci * 128:(ci + 1) * 128, h * HL:(h + 1) * HL])

        # ---- pass 1: x loads on the SP HWDGE queue + l-sum reductions ----
        def load_sample(s, red):
            partials[s] = small.tile([128, CP, NH], F32,
                                     tag=f"partials{s}", name=f"partials{s}")
            for ci in range(CP):
                t = cache.tile([128, L], BF16, tag=f"x{s}{ci}",
                               name=f"x{s}{ci}")
                xt[(s, ci)] = t
                for h in range(NH):
                    ld = nc.sync.dma_start(
                        out=t[:, h * HL:(h + 1) * HL],
                        in_=x_d[s, ci * 128:(ci + 1) * 128,
                                h * HL:(h + 1) * HL])
                    loads.append(ld)
                    if red is not None:
                        reduce_half(s, ci, h, red[ci * NH + h])

        def merge_gmean(s):
            nc.vector.reduce_sum(out=ge[s][:, 1:1 + CP], in_=partials[s][:],
                                 axis=AX_X)

        load_sample(0, S0_RED)
        merge_gmean(0)

        # remaining s1 loads on the SP queue (c2h0 is the tail)
        partials[1] = small.tile([128, CP, NH], F32, tag="partials1",
                                 name="partials1")
        for ci in range(CP - 2):
            t = cache.tile([128, L], BF16, tag=f"x1{ci}", name=f"x1{ci}")
            xt[(1, ci)] = t
        for ci, h in ((0, 0), (0, 1), (1, 0), (1, 1), (2, 0)):
            ld = nc.sync.dma_start(
                out=xt[(1, ci)][:, h * HL:(h + 1) * HL],
                in_=x_d[1, ci * 128:(ci + 1) * 128, h * HL:(h + 1) * HL])
            loads.append(ld)

        # reductions of the early-loaded tail tiles (data lands ~t=15)
        reduce_half(1, 3, 0, 'a')
        reduce_half(1, 3, 1, 'v')
        reduce_half(1, 2, 1, 'v')

        # ---- per-sample gate (hop-minimized) -----------------------------
        def gate_sample(s):
            # PE: MLP layer 1, then all nine constant band-conv matmuls
            hp = psum.tile([HID, 1], F32, tag="hp")
            for i in range(CP):
                nc.tensor.matmul(hp[:], lhsT=cst[:, i * HID:(i + 1) * HID],
                                 rhs=ge[s][:, 1 + i:2 + i],
                                 start=(i == 0), stop=(i == CP - 1))
            yk = []
            for kidx in range(3):
                yp = psum.tile([128, CP], F32, tag=f"y{kidx}")
                nc.tensor.matmul(yp[:], lhsT=wb(0, kidx),
                                 rhs=ge[s][:, 1:1 + CP], start=True,
                                 stop=False)
                nc.tensor.matmul(yp[:], lhsT=wb(1, kidx),
                                 rhs=ge[s][:, 2:2 + CP], start=False,
                                 stop=False)
                nc.tensor.matmul(yp[:], lhsT=wb(2, kidx),
                                 rhs=ge[s][:, 0:CP], start=False, stop=True)
                yk.append(yp)

            h = small.tile([HID, 1], F32, tag="h")
            nc.scalar.activation(h[:], hp[:], ACTF.Gelu, bias=b1, scale=1.0)
            dp = psum.tile([1, 1], F32, tag="dp")
            nc.tensor.matmul(dp[:], lhsT=h[:], rhs=w2t, start=True, stop=True)

            # th = tanh(dp + b2) with the bias fused into the activation
            th = small.tile([1, 1], F32, tag="th")
            nc.scalar.activation(th[:], dp[:], ACTF.Tanh, bias=b2t[:],
                                 scale=1.0)
            # candidate sigmoid halves: tk_k = tanh(y_k / 2)
            tk = []
            for kidx in range(3):
                tt = small.tile([128, CP], F32, tag=f"tk{kidx}")
                nc.scalar.activation(tt[:], yk[kidx][:], ACTF.Tanh, scale=0.5)
                tk.append(tt)

            # broadcast th across partitions, then flags + blend on VectorE
            thp = psum.tile([128, 1], F32, tag="thp")
            nc.tensor.matmul(thp[:], lhsT=ones[:], rhs=th[:], start=True,
                             stop=True)
            fb = small.tile([128, 3], F32, tag="fb")
            nc.vector.tensor_scalar(out=fb[:, 0:1], in0=thp[:], scalar1=0.25,
                                    scalar2=None, op0=ALU.is_ge)
            nc.vector.tensor_scalar(out=fb[:, 1:2], in0=thp[:], scalar1=-0.75,
                                    scalar2=None, op0=ALU.is_lt)
            nc.vector.tensor_add(fb[:, 2:3], fb[:, 0:1], fb[:, 1:2])
            nc.vector.tensor_scalar(out=fb[:, 2:3], in0=fb[:, 2:3],
                                    scalar1=-1.0, scalar2=1.0, op0=ALU.mult,
                                    op1=ALU.add)
            # sg = 0.5 + 0.5*(bb*tk3 + u*tk5 + a*tk7)   (flags sum to 1)
            bl = small.tile([128, CP], F32, tag="bl")
            t2 = small.tile([128, CP], F32, tag="t2")
            nc.vector.tensor_scalar(out=bl[:], in0=tk[0][:],
                                    scalar1=fb[:, 1:2], scalar2=None,
                                    op0=ALU.mult)
            nc.vector.tensor_scalar(out=t2[:], in0=tk[1][:],
                                    scalar1=fb[:, 2:3], scalar2=None,
                                    op0=ALU.mult)
            nc.vector.tensor_add(bl[:], bl[:], t2[:])
            nc.vector.tensor_scalar(out=t2[:], in0=tk[2][:],
                                    scalar1=fb[:, 0:1], scalar2=None,
                                    op0=ALU.mult)
            nc.vector.tensor_add(bl[:], bl[:], t2[:])
            sg = small.tile([128, CP], F32, tag=f"sg{s}")
            nc.vector.tensor_scalar(out=sg[:], in0=bl[:], scalar1=0.5,
                                    scalar2=0.5, op0=ALU.mult, op1=ALU.add)
            return sg

        sg = {}
        sg[0] = gate_sample(0)

        def scale_tile(s, ci):
            t = xt[(s, ci)]
            nc.vector.tensor_scalar_mul(t[:], t[:], sg[s][:, ci:ci + 1])

        def store_tile(s, ci):
            return nc.scalar.dma_start(
                out=o_d[s, ci * 128:(ci + 1) * 128, :],
                in_=xt[(s, ci)][:])

        # ---- s1 reductions / s0 scales / s0 stores in deadline order -----
        reduce_half(1, 0, 0, 'a')
        reduce_half(1, 0, 1, 'a')
        scale_tile(0, 0)
        scale_tile(0, 1)
        reduce_half(1, 1, 0, 'vf')
        reduce_half(1, 1, 1, 'a')
        st0 = store_tile(0, 0)
        add_dep_helper(_inst(st0), _inst(loads[-2]), sync=True,
                       reason="stores after load phase")
        store_tile(0, 1)
        reduce_half(1, 2, 0, 'a')
        merge_gmean(1)
        scale_tile(0, 2)
        store_tile(0, 2)
        scale_tile(0, 3)
        store_tile(0, 3)

        sg[1] = gate_sample(1)
        for ci in range(CP):
            scale_tile(1, ci)
        for ci in range(CP):
            store_tile(1, ci)

    nc.compile()
    return nc


_COMPILED = {}


def _get_compiled(b2_val):
    key = float(b2_val)
    if key not in _COMPILED:
        _COMPILED[key] = _build(key)
    return _COMPILED[key]


def _make_consts(w1, b1, w2, b2):
    w1 = np.asarray(w1, np.float32)
    b1 = np.asarray(b1, np.float32)
    w2 = np.asarray(w2, np.float32)

    S17 = np.zeros((128, 17, 128), np.float32)
    p = np.arange(128)
    for j in range(9):
        d = j - 4
        m = (p + d >= 0) & (p + d < 128)
        S17[p[m] + d, j, p[m]] = 1.0
    for d in range(1, 5):
        m = p + d - 128 >= 0
        S17[p[m] + d - 128, 8 + d, p[m]] = 1.0
    for d in range(-4, 0):
        m = p + d + 128 < 128
        S17[p[m] + d + 128, 17 + d, p[m]] = 1.0

    cst = np.zeros((128, CST_COLS), np.float32)
    # w1t: [CP, 128, HID] flattened as CP blocks of HID columns, 1/L folded
    w1t = (w1.T / np.float32(L)).reshape(CP, 128, HID)
    for i in range(CP):
        cst[:, W1T_OFF + i * HID:W1T_OFF + (i + 1) * HID] = w1t[i]
    j9 = np.arange(9)
    for kidx, k in enumerate((3, 5, 7)):
        w = ((np.abs(j9 - 4) <= (k - 1) // 2).astype(np.float32)
             / np.float32(k) / np.float32(L))
        bands = [sum(w[j] * S17[:, j, :] for j in range(9)),
                 sum(w[d + 4] * S17[:, 8 + d, :] for d in range(1, 5)),
                 sum(w[d + 4] * S17[:, 17 + d, :] for d in range(-4, 0))]
        for m in range(3):
            o = WB_OFF + (m * 3 + kidx) * 128
            cst[:, o:o + 128] = bands[m]
    cst[0:HID, B1_OFF] = b1
    cst[0:HID, W2T_OFF] = w2.reshape(HID)
    return {"cst": np.ascontiguousarray(cst)}


def kernel(x, w1, b1, w2, b2):
    x = np.asarray(x, np.float32)
    assert x.shape == (B, C, L), x.shape
    nc = _get_compiled(np.float32(np.asarray(b2).reshape(-1)[0]))
    consts = _make_consts(w1, b1, w2, b2)
    xb = np.ascontiguousarray(x).astype(ml_dtypes.bfloat16)
    in_maps = []
    for i in range(N_CORES):
        m = {"x": np.ascontiguousarray(xb[i * BS:(i + 1) * BS])}
        m.update(consts)
        in_maps.append(m)
    res = run_bass_kernel_spmd(nc, in_maps, list(range(N_CORES)),
                               trace=bool(int(os.environ.get("K_TRACE", "0"))))
    out = np.concatenate(
        [np.asarray(res.results[i]["out"]).astype(np.float32)
         for i in range(N_CORES)], axis=0)
    if res.exec_time_ns is not None:
        kernel.last_exec_time_ns = res.exec_time_ns
        kernel.last_mean_exec_time_ns = res.mean_exec_time_ns
    kernel.last_results = res
    return out


# revision 17
# speedup vs baseline: 1.0296x; 1.0296x over previous
"""Trainium2 Bass kernel for the dynamic-kernel ECA module.

Computation per sample:
  gap  = mean(x, axis=l)                       (c,)
  h    = gelu(gap @ w1.T + b1)                 (hidden,)
  th   = tanh(h @ w2.T + b2); delta = 2*th     scalar
  k    = (5 + clip(round(delta), -3, 3)) | 1   in {3,5,7} (delta in (-2,2))
  w    = box filter of width k in 9-tap window, 1/k weights
  y    = conv1d(gap, w) along c (zero pad 4)   (c,)
  s    = sigmoid(y)
  out  = x * s[:, None]

Sharding: pure data parallel, batch 16 -> 8 cores x 2 samples.

Memory strategy (per core): x moves through HBM in bf16 (the 2e-2
rel-err budget gives ~10x margin over bf16's ~2e-3 quantization; the
kernel is purely HBM-bandwidth-bound at ~420 GB/s/core).  The 16 MiB
bf16 shard fits in SBUF entirely, so x is read exactly once and out
written exactly once (32 MiB/core of traffic ~= 80 us of DMA).

Queue discipline: the SP HWDGE queue carries ONLY the 16 x-tile loads
(an in-order sequencer; any gate-dependent wait interleaved there
head-of-line-blocks the remaining loads).  All constants ship as a
single packed transfer on the ScalarE HWDGE queue, which also carries
the 8 stores.  The first store is gated on the third-to-last load
(completion receipts lag data by a few us under full DMA load), giving
a pure read phase then a pure write phase.

The gate is built to minimize serial cross-engine hops, which at ~1 us
each (sem latency + engine-stream contention) dominate its latency:
PE runs the MLP matmuls AND all nine candidate band-conv matmuls
(k in {3,5,7} x in-chunk/hi-wrap/lo-wrap, host-precomputed constant
lhsT) back to back; ScalarE turns each candidate PSUM into
tanh(y_k/2); the scalar th is broadcast across partitions by a
ones-row matmul; VectorE then computes the three k-selection flags and
blends the three candidate results (sigmoid(y) = 0.5 + 0.5*tanh(y/2),
and sum(flags) = 1, so the affine folds into one op).  Nothing
data-dependent ever touches the matmul weights.

Load reductions (l-sums) are split per 1 MiB half-load and spread over
VectorE/ScalarE by explicit deadline order: sample-1's tail reductions
gate gate1 -> s1 scales -> s1 stores, so they get dedicated slots;
VectorE-assigned ones use a bf16 2x-mode fold-add before a half-size
1x reduce.  1/L is folded into w1 and the band weights on the host.
"""

import os
from contextlib import ExitStack

import numpy as np
import ml_dtypes

import concourse.bacc as bacc
import concourse.mybir as mybir
import concourse.tile as tile
from concourse.tile_rust import add_dep_helper
from concourse.bass_utils import run_bass_kernel_spmd

F32 = mybir.dt.float32
BF16 = mybir.dt.bfloat16
ALU = mybir.AluOpType
ACTF = mybir.ActivationFunctionType
AX_X = mybir.AxisListType.X

B, C, L = 16, 512, 8192
HID = 64
N_CORES = 8
BS = B // N_CORES            # samples per core = 2
CP = C // 128                # channel chunks = 4
NH = 2                       # load halves per tile
HL = L // NH                 # 4096 elements = 1 MiB bf16 per half-load

# packed const blob layout (f32 columns)
W1T_OFF = 0                  # [128, CP*HID] = 256 cols
WB_OFF = 256                 # [128, 3*3*128] = 1152 cols (m-major, k-minor)
B1_OFF = 1408                # [64, 1]
W2T_OFF = 1409               # [64, 1]
CST_COLS = 1410

S0_RED = ['v', 'a', 'v', 'a', 'v', 'a', 'v', 'a']


def _inst(x):
    return getattr(x, "ins", x)


def _build(b2_val):
    nc = bacc.Bacc("TRN2", target_bir_lowering=False, debug=False,
                   num_devices=N_CORES)

    x_d = nc.dram_tensor("x", [BS, C, L], BF16, kind="ExternalInput").ap()
    cst_d = nc.dram_tensor("cst", [128, CST_COLS], F32,
                           kind="ExternalInput").ap()
    o_d = nc.dram_tensor("out", [BS, C, L], BF16, kind="ExternalOutput").ap()

    with ExitStack() as ctx:
        tc = ctx.enter_context(tile.TileContext(nc))
        cache = ctx.enter_context(tc.tile_pool(name="cache", bufs=1))
        small = ctx.enter_context(tc.tile_pool(name="small", bufs=1))
        psum = ctx.enter_context(tc.tile_pool(name="psum", bufs=1, space="PSUM"))

        xt = {}
        partials = {}
        ge = {}
        loads = []

        def wb(m, kidx):
            o = WB_OFF + (m * 3 + kidx) * 128
            return cst[:, o:o + 128]

        def reduce_half(s, ci, h, eng):
            t = xt[(s, ci)][:, h * HL:(h + 1) * HL]
            dst = partials[s][:, ci, h:h + 1]
            if eng == 'v':
                nc.vector.reduce_sum(out=dst, in_=t, axis=AX_X)
            elif eng == 'vf':
                # bf16 fold-add at 2x mode, then a half-size 1x reduce
                f = small.tile([128, HL // 2], BF16, tag="fold")
                nc.vector.tensor_add(f[:], t[:, 0:HL // 2], t[:, HL // 2:HL])
                nc.vector.reduce_sum(out=dst, in_=f[:], axis=AX_X)
            else:
                nc.scalar.activation(t, t, ACTF.Copy, accum_out=dst)

        # ---- init + packed consts (single ScalarE-queue transfer) --------
        ge[0] = small.tile([128, CP + 2], F32, tag="ge0", name="ge0")
        ge[1] = small.tile([128, CP + 2], F32, tag="ge1", name="ge1")
        ones = small.tile([1, 128], F32, tag="ones")
        b2t = small.tile([1, 1], F32, tag="b2t")
        nc.vector.memset(ge[0][:], 0.0)
        nc.vector.memset(ge[1][:], 0.0)
        nc.vector.memset(ones[:], 1.0)
        nc.vector.memset(b2t[:], float(b2_val))
        cst = small.tile([128, CST_COLS], F32, tag="cst")
        nc.scalar.dma_start(out=cst[:], in_=cst_d[:])
        b1 = cst[0:HID, B1_OFF:B1_OFF + 1]
        w2t = cst[0:HID, W2T_OFF:W2T_OFF + 1]

        # ---- pass 1: x loads on the SP HWDGE queue + l-sum reductions ----
        def load_sample(s, red):
            partials[s] = small.tile([128, CP, NH], F32,
                                     tag=f"partials{s}", name=f"partials{s}")
            for ci in range(CP):
                t = cache.tile([128, L], BF16, tag=f"x{s}{ci}",
                               name=f"x{s}{ci}")
                xt[(s, ci)] = t
                for h in range(NH):
                    ld = nc.sync.dma_start(
                        out=t[:, h * HL:(h + 1) * HL],
                        in_=x_d[s, ci * 128:(ci + 1) * 128,
                                h * HL:(h + 1) * HL])
                    loads.append(ld)
                    if red is not None:
                        reduce_half(s, ci, h, red[ci * NH + h])

        def merge_gmean(s):
            nc.vector.reduce_sum(out=ge[s][:, 1:1 + CP], in_=partials[s][:],
                                 axis=AX_X)

        load_sample(0, S0_RED)
        merge_gmean(0)
        load_sample(1, None)

        # ---- per-sample gate (hop-minimized) -----------------------------
        def gate_sample(s):
            # PE: MLP layer 1, then all nine constant band-conv matmuls
            hp = psum.tile([HID, 1], F32, tag="hp")
            for i in range(CP):
                nc.tensor.matmul(hp[:], lhsT=cst[:, i * HID:(i + 1) * HID],
                                 rhs=ge[s][:, 1 + i:2 + i],
                                 start=(i == 0), stop=(i == CP - 1))
            yk = []
            for kidx in range(3):
                yp = psum.tile([128, CP], F32, tag=f"y{kidx}")
                nc.tensor.matmul(yp[:], lhsT=wb(0, kidx),
                                 rhs=ge[s][:, 1:1 + CP], start=True,
                                 stop=False)
                nc.tensor.matmul(yp[:], lhsT=wb(1, kidx),
                                 rhs=ge[s][:, 2:2 + CP], start=False,
                                 stop=False)
                nc.tensor.matmul(yp[:], lhsT=wb(2, kidx),
                                 rhs=ge[s][:, 0:CP], start=False, stop=True)
                yk.append(yp)

            h = small.tile([HID, 1], F32, tag="h")
            nc.scalar.activation(h[:], hp[:], ACTF.Gelu, bias=b1, scale=1.0)
            dp = psum.tile([1, 1], F32, tag="dp")
            nc.tensor.matmul(dp[:], lhsT=h[:], rhs=w2t, start=True, stop=True)

            # th = tanh(dp + b2) with the bias fused into the activation
            th = small.tile([1, 1], F32, tag="th")
            nc.scalar.activation(th[:], dp[:], ACTF.Tanh, bias=b2t[:],
                                 scale=1.0)
            # candidate sigmoid halves: tk_k = tanh(y_k / 2)
            tk = []
            for kidx in range(3):
                tt = small.tile([128, CP], F32, tag=f"tk{kidx}")
                nc.scalar.activation(tt[:], yk[kidx][:], ACTF.Tanh, scale=0.5)
                tk.append(tt)

            # broadcast th across partitions, then flags + blend on VectorE
            thp = psum.tile([128, 1], F32, tag="thp")
            nc.tensor.matmul(thp[:], lhsT=ones[:], rhs=th[:], start=True,
                             stop=True)
            fb = small.tile([128, 3], F32, tag="fb")
            nc.vector.tensor_scalar(out=fb[:, 0:1], in0=thp[:], scalar1=0.25,
                                    scalar2=None, op0=ALU.is_ge)
            nc.vector.tensor_scalar(out=fb[:, 1:2], in0=thp[:], scalar1=-0.75,
                                    scalar2=None, op0=ALU.is_lt)
            nc.vector.tensor_add(fb[:, 2:3], fb[:, 0:1], fb[:, 1:2])
            nc.vector.tensor_scalar(out=fb[:, 2:3], in0=fb[:, 2:3],
                                    scalar1=-1.0, scalar2=1.0, op0=ALU.mult,
                                    op1=ALU.add)
            # sg = 0.5 + 0.5*(bb*tk3 + u*tk5 + a*tk7)   (flags sum to 1)
            bl = small.tile([128, CP], F32, tag="bl")
            t2 = small.tile([128, CP], F32, tag="t2")
            nc.vector.tensor_scalar(out=bl[:], in0=tk[0][:],
                                    scalar1=fb[:, 1:2], scalar2=None,
                                    op0=ALU.mult)
            nc.vector.tensor_scalar(out=t2[:], in0=tk[1][:],
                                    scalar1=fb[:, 2:3], scalar2=None,
                                    op0=ALU.mult)
            nc.vector.tensor_add(bl[:], bl[:], t2[:])
            nc.vector.tensor_scalar(out=t2[:], in0=tk[2][:],
                                    scalar1=fb[:, 0:1], scalar2=None,
                                    op0=ALU.mult)
            nc.vector.tensor_add(bl[:], bl[:], t2[:])
            sg = small.tile([128, CP], F32, tag=f"sg{s}")
            nc.vector.tensor_scalar(out=sg[:], in0=bl[:], scalar1=0.5,
                                    scalar2=0.5, op0=ALU.mult, op1=ALU.add)
            return sg

        sg = {}
        sg[0] = gate_sample(0)

        def scale_tile(s, ci):
            t = xt[(s, ci)]
            nc.vector.tensor_scalar_mul(t[:], t[:], sg[s][:, ci:ci + 1])

        def store_tile(s, ci):
            return nc.scalar.dma_start(
                out=o_d[s, ci * 128:(ci + 1) * 128, :],
                in_=xt[(s, ci)][:])

        # ---- s1 reductions / s0 scales / s0 stores in deadline order -----
        # ScalarE: reds 0,1,3,5,6,7 (+ store triggers slotted between);
        # VectorE: folds for reds 2,4 between the s0 scale muls.
        reduce_half(1, 0, 0, 'a')
        reduce_half(1, 0, 1, 'a')
        scale_tile(0, 0)
        reduce_half(1, 1, 0, 'vf')
        reduce_half(1, 1, 1, 'a')
        scale_tile(0, 1)
        reduce_half(1, 2, 0, 'vf')
        reduce_half(1, 2, 1, 'a')
        st0 = store_tile(0, 0)
        add_dep_helper(_inst(st0), _inst(loads[-3]), sync=True,
                       reason="stores after load phase")
        reduce_half(1, 3, 0, 'a')
        store_tile(0, 1)
        reduce_half(1, 3, 1, 'a')
        merge_gmean(1)
        scale_tile(0, 2)
        store_tile(0, 2)
        scale_tile(0, 3)
        store_tile(0, 3)

        sg[1] = gate_sample(1)
        for ci in range(CP):
            scale_tile(1, ci)
        for ci in range(CP):
            store_tile(1, ci)

    nc.compile()
    return nc


_COMPILED = {}


def _get_compiled(b2_val):
    key = float(b2_val)
    if key not in _COMPILED:
        _COMPILED[key] = _build(key)
    return _COMPILED[key]


def _make_consts(w1, b1, w2, b2):
    w1 = np.asarray(w1, np.float32)
    b1 = np.asarray(b1, np.float32)
    w2 = np.asarray(w2, np.float32)

    S17 = np.zeros((128, 17, 128), np.float32)
    p = np.arange(128)
    for j in range(9):
        d = j - 4
        m = (p + d >= 0) & (p + d < 128)
        S17[p[m] + d, j, p[m]] = 1.0
    for d in range(1, 5):
        m = p + d - 128 >= 0
        S17[p[m] + d - 128, 8 + d, p[m]] = 1.0
    for d in range(-4, 0):
        m = p + d + 128 < 128
        S17[p[m] + d + 128, 17 + d, p[m]] = 1.0

    cst = np.zeros((128, CST_COLS), np.float32)
    # w1t: [CP, 128, HID] flattened as CP blocks of HID columns, 1/L folded
    w1t = (w1.T / np.float32(L)).reshape(CP, 128, HID)
    for i in range(CP):
        cst[:, W1T_OFF + i * HID:W1T_OFF + (i + 1) * HID] = w1t[i]
    j9 = np.arange(9)
    for kidx, k in enumerate((3, 5, 7)):
        w = ((np.abs(j9 - 4) <= (k - 1) // 2).astype(np.float32)
             / np.float32(k) / np.float32(L))
        bands = [sum(w[j] * S17[:, j, :] for j in range(9)),
                 sum(w[d + 4] * S17[:, 8 + d, :] for d in range(1, 5)),
                 sum(w[d + 4] * S17[:, 17 + d, :] for d in range(-4, 0))]
        for m in range(3):
            o = WB_OFF + (m * 3 + kidx) * 128
            cst[:, o:o + 128] = bands[m]
    cst[0:HID, B1_OFF] = b1
    cst[0:HID, W2T_OFF] = w2.reshape(HID)
    return {"cst": np.ascontiguousarray(cst)}


def kernel(x, w1, b1, w2, b2):
    x = np.asarray(x, np.float32)
    assert x.shape == (B, C, L), x.shape
    nc = _get_compiled(np.float32(np.asarray(b2).reshape(-1)[0]))
    consts = _make_consts(w1, b1, w2, b2)
    xb = np.ascontiguousarray(x).astype(ml_dtypes.bfloat16)
    in_maps = []
    for i in range(N_CORES):
        m = {"x": np.ascontiguousarray(xb[i * BS:(i + 1) * BS])}
        m.update(consts)
        in_maps.append(m)
    res = run_bass_kernel_spmd(nc, in_maps, list(range(N_CORES)),
                               trace=bool(int(os.environ.get("K_TRACE", "0"))))
    out = np.concatenate(
        [np.asarray(res.results[i]["out"]).astype(np.float32)
         for i in range(N_CORES)], axis=0)
    if res.exec_time_ns is not None:
        kernel.last_exec_time_ns = res.exec_time_ns
        kernel.last_mean_exec_time_ns = res.mean_exec_time_ns
    kernel.last_results = res
    return out
